# revision 3
# baseline (speedup 1.0000x reference)
"""Trainium2 Bass kernel for nn_JetBlock (pre-LN attention + SwiGLU MLP).

Sharding: 8 cores = (batch b in {0,1}) x (sequence chunk i in {0..3}, 512
tokens). Each core computes its own 512-token output chunk end-to-end:
 - LN1 over the full 2048-token batch (needed for K/V), projections Q
   (own tokens) and K/V (all tokens), RoPE, causal softmax via a 0/1
   multiplicative mask, attention, o-proj + residual, LN2, SwiGLU MLP +
   residual.  No cross-core communication (K/V projection is replicated
   within a batch group), so the SPMD program is identical on all cores
   and only the per-core input data differs.
All matmuls run in bf16 with fp32 PSUM accumulation.
"""

import os
import sys
import time

sys.path.insert(0, "/opt/trn_rl_repo")

import numpy as np
import ml_dtypes

import concourse.bass as bass
import concourse.mybir as mybir
import concourse.tile as tile
from concourse import bacc
from concourse.bass_utils import run_bass_kernel_spmd

BF16 = mybir.dt.bfloat16
F32 = mybir.dt.float32
AX = mybir.AxisListType
OP = mybir.AluOpType
AF = mybir.ActivationFunctionType

B, T, C, H = 2, 2048, 2048, 16
DH = C // H          # 128
INNER = 4 * C        # 8192
EPS = 1e-5
NCORES = 8
CHUNK = T // 4       # 512 tokens per core
P = 128

nbf = ml_dtypes.bfloat16

_CACHE = {}
LAST_RESULTS = None


# --------------------------------------------------------------------------
# program builder
# --------------------------------------------------------------------------

def _build_program():
    nc = bacc.Bacc("TRN2", target_bir_lowering=False, debug=False,
                   num_devices=NCORES)

    # external inputs (per-core data)
    xb = nc.dram_tensor("xb", [T, C], F32, kind="ExternalInput").ap()
    xown = nc.dram_tensor("xown", [CHUNK, C], F32, kind="ExternalInput").ap()
    wq = nc.dram_tensor("wq", [C, C], BF16, kind="ExternalInput").ap()
    wkv = nc.dram_tensor("wkv", [C, 2 * C], BF16, kind="ExternalInput").ap()
    wo = nc.dram_tensor("wo", [C, C], BF16, kind="ExternalInput").ap()
    w1 = nc.dram_tensor("w1", [C, INNER], BF16, kind="ExternalInput").ap()
    w2 = nc.dram_tensor("w2", [C, INNER], BF16, kind="ExternalInput").ap()
    w3 = nc.dram_tensor("w3", [INNER, C], BF16, kind="ExternalInput").ap()
    cos_own = nc.dram_tensor("cos_own", [CHUNK, 64], F32, kind="ExternalInput").ap()
    sin_own = nc.dram_tensor("sin_own", [CHUNK, 64], F32, kind="ExternalInput").ap()
    cos_all = nc.dram_tensor("cos_all", [T, 64], F32, kind="ExternalInput").ap()
    sin_all = nc.dram_tensor("sin_all", [T, 64], F32, kind="ExternalInput").ap()
    maskT = nc.dram_tensor("maskT", [T, CHUNK], BF16, kind="ExternalInput").ap()
    b1v = nc.dram_tensor("b1v", [P, INNER // P], F32, kind="ExternalInput").ap()
    b2v = nc.dram_tensor("b2v", [P, INNER // P], F32, kind="ExternalInput").ap()

    # internal DRAM scratch
    xln_d = nc.dram_tensor("xln_d", [T, C], BF16).ap()
    xownln_d = nc.dram_tensor("xownln_d", [CHUNK, C], BF16).ap()
    q_d = nc.dram_tensor("q_d", [CHUNK, C], BF16).ap()
    k_d = nc.dram_tensor("k_d", [T, C], BF16).ap()
    v_d = nc.dram_tensor("v_d", [T, H, 132], BF16).ap()
    y_d = nc.dram_tensor("y_d", [CHUNK, C], BF16).ap()
    x1ln_d = nc.dram_tensor("x1ln_d", [CHUNK, C], BF16).ap()

    out = nc.dram_tensor("out", [CHUNK, C], F32, kind="ExternalOutput").ap()

    # rearranged DRAM views
    xb_r = xb.rearrange("(to ti) c -> ti to c", ti=P)        # [128,16,C]
    xown_r = xown.rearrange("(qo qi) c -> qi qo c", qi=P)    # [128,4,C]
    wq_r = wq.rearrange("(co ci) f -> ci co f", ci=P)
    wkv_r = wkv.rearrange("(co ci) f -> ci co f", ci=P)
    wo_r = wo.rearrange("(co ci) f -> ci co f", ci=P)
    w1_r = w1.rearrange("(co ci) f -> ci co f", ci=P)
    w2_r = w2.rearrange("(co ci) f -> ci co f", ci=P)
    w3_r = w3.rearrange("(ko ki) f -> ki ko f", ki=P)
    cos_own_r = cos_own.rearrange("(qo qi) j -> qi qo j", qi=P)  # [128,4,64]
    sin_own_r = sin_own.rearrange("(qo qi) j -> qi qo j", qi=P)
    cos_all_r = cos_all.rearrange("(to ti) j -> ti to j", ti=P)  # [128,16,64]
    sin_all_r = sin_all.rearrange("(to ti) j -> ti to j", ti=P)
    maskT_r = maskT.rearrange("(ko ki) q -> ki ko q", ki=P)      # [128,16,512]
    out_r = out.rearrange("(qo qi) c -> qi qo c", qi=P)

    TTILES = T // P      # 16
    QTILES = CHUNK // P  # 4
    CC = C // P          # 16 contraction chunks over C
    KC = INNER // P      # 64 contraction chunks over INNER

    with tile.TileContext(nc) as tc:
        # persistent pools
        with (
            tc.tile_pool(name="px1", bufs=1) as px1,
            tc.tile_pool(name="small", bufs=4) as small,
            tc.tile_pool(name="psA", bufs=5, space="PSUM") as psA,
            tc.tile_pool(name="psB", bufs=3, space="PSUM") as psB,
        ):
            x1 = px1.tile([P, QTILES, C], F32, tag="x1")

            def layernorm(pool, src_ap, n_tiles, dst_dram, dst_rows_per_tile):
                """LN over C for token-major tiles; writes bf16 to DRAM."""
                for tt in range(n_tiles):
                    xt = pool.tile([P, C], F32, tag="ln_x", bufs=2)
                    nc.sync.dma_start(xt[:], src_ap[:, tt, :])
                    s = small.tile([P, 1], F32, tag="ln_s")
                    nc.vector.reduce_sum(s[:], xt[:], axis=AX.X)
                    negmu = small.tile([P, 1], F32, tag="ln_negmu")
                    nc.vector.tensor_scalar_mul(negmu[:], s[:], -1.0 / C)
                    sq = pool.tile([P, C], F32, tag="ln_sq", bufs=2)
                    vsum = small.tile([P, 1], F32, tag="ln_vsum")
                    nc.scalar.activation(sq[:], xt[:], AF.Square,
                                         bias=negmu[:], scale=1.0,
                                         accum_out=vsum[:])
                    var = small.tile([P, 1], F32, tag="ln_var")
                    nc.vector.tensor_scalar(var[:], vsum[:], 1.0 / C, EPS,
                                            op0=OP.mult, op1=OP.add)
                    std = small.tile([P, 1], F32, tag="ln_std")
                    nc.scalar.sqrt(std[:], var[:])
                    rstd = small.tile([P, 1], F32, tag="ln_rstd")
                    nc.vector.reciprocal(rstd[:], std[:])
                    o = pool.tile([P, C], BF16, tag="ln_o", bufs=2)
                    nc.vector.tensor_scalar(o[:], xt[:], negmu[:], rstd[:],
                                            op0=OP.add, op1=OP.mult)
                    nc.sync.dma_start(
                        dst_dram[tt * dst_rows_per_tile:(tt + 1) * dst_rows_per_tile, :]
                        .rearrange("(a p) c -> p a c", p=P),
                        o[:, None, :])

            def rope_evict(ps, stage, cos_sb, sin_sb, tt):
                """RoPE on a [128, 512] psum holding 4 heads; writes bf16 stage."""
                pse = ps[:].rearrange("p (h j two) -> p h j two", h=4, two=2)
                ste = stage[:].rearrange("p (h j two) -> p h j two", h=4, two=2)
                cosb = cos_sb[:, tt, None, :].to_broadcast([P, 4, 64])
                sinb = sin_sb[:, tt, None, :].to_broadcast([P, 4, 64])
                e = pse[:, :, :, 0]
                o_ = pse[:, :, :, 1]
                t1 = small.tile([P, 4, 64], F32, tag="rope_t1")
                t2 = small.tile([P, 4, 64], F32, tag="rope_t2")
                nc.vector.tensor_mul(t1[:], e, cosb)
                nc.vector.tensor_mul(t2[:], o_, sinb)
                nc.vector.tensor_sub(ste[:, :, :, 0], t1[:], t2[:])
                nc.vector.tensor_mul(t1[:], e, sinb)
                nc.vector.tensor_mul(t2[:], o_, cosb)
                nc.vector.tensor_add(ste[:, :, :, 1], t1[:], t2[:])

            with tc.tile_pool(name="pA", bufs=1) as pA:
                # ---------------- phase A: LN1 ----------------
                layernorm(pA, xb_r, TTILES, xln_d, P)
                layernorm(pA, xown_r, QTILES, xownln_d, P)
                # transpose-load LN results to feature-major
                xlnT = pA.tile([P, CC, T], BF16, tag="xlnT")
                for co in range(CC):
                    nc.sync.dma_start(xlnT[:, co, :], xln_d[:, co * P:(co + 1) * P],
                                      transpose=True)
                xownlnT = pA.tile([P, CC, CHUNK], BF16, tag="xownlnT")
                for co in range(CC):
                    nc.sync.dma_start(xownlnT[:, co, :],
                                      xownln_d[:, co * P:(co + 1) * P],
                                      transpose=True)
                cos_own_sb = pA.tile([P, QTILES, 64], F32, tag="cos_own")
                sin_own_sb = pA.tile([P, QTILES, 64], F32, tag="sin_own")
                cos_all_sb = pA.tile([P, TTILES, 64], F32, tag="cos_all")
                sin_all_sb = pA.tile([P, TTILES, 64], F32, tag="sin_all")
                nc.sync.dma_start(cos_own_sb[:], cos_own_r)
                nc.sync.dma_start(sin_own_sb[:], sin_own_r)
                nc.sync.dma_start(cos_all_sb[:], cos_all_r)
                nc.sync.dma_start(sin_all_sb[:], sin_all_r)

                # ---------------- phase C: Q projection (own tokens) --------
                for ft in range(4):
                    pss = [psA.tile([P, 512], F32, tag="acc", name=f"ps_q{ft}_{j}") for j in range(QTILES)]
                    for cc in range(CC):
                        wblk = pA.tile([P, 512], BF16, tag="w_qo", bufs=4)
                        nc.sync.dma_start(wblk[:], wq_r[:, cc, ft * 512:(ft + 1) * 512])
                        for qt in range(QTILES):
                            nc.tensor.matmul(pss[qt][:],
                                             xownlnT[:, cc, qt * P:(qt + 1) * P],
                                             wblk[:],
                                             start=(cc == 0), stop=(cc == CC - 1))
                    for qt in range(QTILES):
                        stage = pA.tile([P, 512], BF16, tag="qk_stage", bufs=3)
                        rope_evict(pss[qt], stage, cos_own_sb, sin_own_sb, qt)
                        nc.sync.dma_start(
                            q_d[qt * P:(qt + 1) * P, ft * 512:(ft + 1) * 512],
                            stage[:])

                # ---------------- phase D: K/V projection (all tokens) ------
                for tg in range(4):
                    for ft in range(8):
                        pss = [psA.tile([P, 512], F32, tag="acc", name=f"ps_kv{tg}_{ft}_{j}") for j in range(4)]
                        for cc in range(CC):
                            wblk = pA.tile([P, 512], BF16, tag="w_kv", bufs=4)
                            nc.sync.dma_start(wblk[:],
                                              wkv_r[:, cc, ft * 512:(ft + 1) * 512])
                            for tt in range(4):
                                nc.tensor.matmul(
                                    pss[tt][:],
                                    xlnT[:, cc, (tg * 4 + tt) * P:(tg * 4 + tt + 1) * P],
                                    wblk[:],
                                    start=(cc == 0), stop=(cc == CC - 1))
                        for tt in range(4):
                            gt = tg * 4 + tt
                            if ft < 4:   # K part + RoPE
                                stage = pA.tile([P, 512], BF16, tag="qk_stage", bufs=3)
                                rope_evict(pss[tt], stage, cos_all_sb, sin_all_sb, gt)
                                nc.sync.dma_start(
                                    k_d[gt * P:(gt + 1) * P, ft * 512:(ft + 1) * 512],
                                    stage[:])
                            else:        # V part (+ ones column for denominator)
                                hg = ft - 4
                                stage = pA.tile([P, 4, 132], BF16, tag="v_stage", bufs=3)
                                nc.scalar.copy(
                                    stage[:, :, 0:P],
                                    pss[tt][:].rearrange("p (h d) -> p h d", h=4))
                                nc.vector.memset(stage[:, :, P:132], 1.0)
                                nc.sync.dma_start(
                                    v_d[gt * P:(gt + 1) * P, hg * 4:(hg + 1) * 4, :],
                                    stage[:])

            # ---------------- phase E/F: attention ----------------
            with tc.tile_pool(name="pE", bufs=1) as pE:
                qT = pE.tile([P, H, CHUNK], BF16, tag="qT")
                kT = pE.tile([P, H, T], BF16, tag="kT")
                for h in range(H):
                    nc.sync.dma_start(qT[:, h, :], q_d[:, h * P:(h + 1) * P],
                                      transpose=True)
                    nc.sync.dma_start(kT[:, h, :], k_d[:, h * P:(h + 1) * P],
                                      transpose=True)
                mask_sb = pE.tile([P, TTILES, CHUNK], BF16, tag="mask")
                nc.sync.dma_start(mask_sb[:], maskT_r)
                y_sb = pE.tile([P, QTILES, C], BF16, tag="y")

                for h in range(H):
                    expS = pE.tile([P, TTILES, CHUNK], BF16, tag="expS", bufs=2)
                    for kc in range(TTILES):
                        ps_s = psB.tile([P, CHUNK], F32, tag="mm")
                        nc.tensor.matmul(ps_s[:], kT[:, h, kc * P:(kc + 1) * P],
                                         qT[:, h, :], start=True, stop=True)
                        nc.scalar.activation(expS[:, kc, :], ps_s[:], AF.Exp)
                    nc.vector.tensor_mul(expS[:], expS[:], mask_sb[:])
                    for qt in range(QTILES):
                        ps_z = psA.tile([P, 132], F32, tag="acc")
                        for kc in range(TTILES):
                            vr = small.tile([P, 132], BF16, tag="v_rhs")
                            nc.sync.dma_start(vr[:], v_d[kc * P:(kc + 1) * P, h, :])
                            nc.tensor.matmul(ps_z[:],
                                             expS[:, kc, qt * P:(qt + 1) * P],
                                             vr[:],
                                             start=(kc == 0), stop=(kc == TTILES - 1))
                        den = small.tile([P, 1], F32, tag="den")
                        nc.vector.reciprocal(den[:], ps_z[:, P:P + 1])
                        nc.vector.tensor_scalar(y_sb[:, qt, h * P:(h + 1) * P],
                                                ps_z[:, 0:P], den[:], None,
                                                op0=OP.mult)
                for qt in range(QTILES):
                    nc.sync.dma_start(
                        y_d[qt * P:(qt + 1) * P, :].rearrange("(a p) c -> p a c", p=P),
                        y_sb[:, qt, None, :])

            # ---------------- phase H: o-proj + residual ----------------
            with tc.tile_pool(name="pH", bufs=1) as pH:
                yT = pH.tile([P, CC, CHUNK], BF16, tag="yT")
                for co in range(CC):
                    nc.sync.dma_start(yT[:, co, :], y_d[:, co * P:(co + 1) * P],
                                      transpose=True)
                xown_sb = pH.tile([P, QTILES, C], F32, tag="xown")
                nc.sync.dma_start(xown_sb[:], xown_r)
                for ft in range(4):
                    pss = [psA.tile([P, 512], F32, tag="acc", name=f"ps_o{ft}_{j}") for j in range(QTILES)]
                    for cc in range(CC):
                        wblk = pH.tile([P, 512], BF16, tag="w_o", bufs=4)
                        nc.sync.dma_start(wblk[:], wo_r[:, cc, ft * 512:(ft + 1) * 512])
                        for qt in range(QTILES):
                            nc.tensor.matmul(pss[qt][:],
                                             yT[:, cc, qt * P:(qt + 1) * P],
                                             wblk[:],
                                             start=(cc == 0), stop=(cc == CC - 1))
                    for qt in range(QTILES):
                        nc.vector.tensor_add(x1[:, qt, ft * 512:(ft + 1) * 512],
                                             pss[qt][:],
                                             xown_sb[:, qt, ft * 512:(ft + 1) * 512])

            # ---------------- phase I: LN2 ----------------
            with tc.tile_pool(name="pI", bufs=1) as pI:
              for qt in range(QTILES):
                xt = x1[:, qt, :]
                s = small.tile([P, 1], F32, tag="ln_s")
                nc.vector.reduce_sum(s[:], xt, axis=AX.X)
                negmu = small.tile([P, 1], F32, tag="ln_negmu")
                nc.vector.tensor_scalar_mul(negmu[:], s[:], -1.0 / C)
                sq = pI.tile([P, C], F32, tag="ln_sq", bufs=2)
                vsum = small.tile([P, 1], F32, tag="ln_vsum")
                nc.scalar.activation(sq[:], xt, AF.Square, bias=negmu[:],
                                     scale=1.0, accum_out=vsum[:])
                var = small.tile([P, 1], F32, tag="ln_var")
                nc.vector.tensor_scalar(var[:], vsum[:], 1.0 / C, EPS,
                                        op0=OP.mult, op1=OP.add)
                std = small.tile([P, 1], F32, tag="ln_std")
                nc.scalar.sqrt(std[:], var[:])
                rstd = small.tile([P, 1], F32, tag="ln_rstd")
                nc.vector.reciprocal(rstd[:], std[:])
                o = pI.tile([P, C], BF16, tag="ln_o", bufs=2)
                nc.vector.tensor_scalar(o[:], xt, negmu[:], rstd[:],
                                        op0=OP.add, op1=OP.mult)
                nc.sync.dma_start(
                    x1ln_d[qt * P:(qt + 1) * P, :].rearrange("(a p) c -> p a c", p=P),
                    o[:, None, :])

            # ---------------- phase J/K: SwiGLU MLP ----------------
            with tc.tile_pool(name="pJ", bufs=1) as pJ:
                x1lnT = pJ.tile([P, CC, CHUNK], BF16, tag="x1lnT")
                for co in range(CC):
                    nc.sync.dma_start(x1lnT[:, co, :], x1ln_d[:, co * P:(co + 1) * P],
                                      transpose=True)
                b1_sb = pJ.tile([P, INNER // P], F32, tag="b1")
                b2_sb = pJ.tile([P, INNER // P], F32, tag="b2")
                nc.sync.dma_start(b1_sb[:], b1v)
                nc.sync.dma_start(b2_sb[:], b2v)
                g = pJ.tile([P, KC, CHUNK], BF16, tag="g")
                for m in range(KC):
                    w1b = pJ.tile([P, CC, P], BF16, tag="w1b", bufs=3)
                    nc.sync.dma_start(w1b[:], w1_r[:, :, m * P:(m + 1) * P])
                    w2b = pJ.tile([P, CC, P], BF16, tag="w2b", bufs=3)
                    nc.sync.dma_start(w2b[:], w2_r[:, :, m * P:(m + 1) * P])
                    ps1 = psB.tile([P, CHUNK], F32, tag="mm")
                    for cc in range(CC):
                        nc.tensor.matmul(ps1[:], w1b[:, cc, :], x1lnT[:, cc, :],
                                         start=(cc == 0), stop=(cc == CC - 1))
                    h1s = pJ.tile([P, CHUNK], BF16, tag="h1s", bufs=3)
                    nc.scalar.activation(h1s[:], ps1[:], AF.Silu,
                                         bias=b1_sb[:, m:m + 1], scale=1.0)
                    ps2 = psB.tile([P, CHUNK], F32, tag="mm")
                    for cc in range(CC):
                        nc.tensor.matmul(ps2[:], w2b[:, cc, :], x1lnT[:, cc, :],
                                         start=(cc == 0), stop=(cc == CC - 1))
                    nc.vector.scalar_tensor_tensor(g[:, m, :], ps2[:],
                                                   b2_sb[:, m:m + 1], h1s[:],
                                                   op0=OP.add, op1=OP.mult)
                # W3: out = x1 + g @ W3
                for ft in range(4):
                    pss = [psA.tile([P, 512], F32, tag="acc", name=f"ps_m{ft}_{j}") for j in range(QTILES)]
                    for kc in range(KC):
                        w3b = pJ.tile([P, 512], BF16, tag="w3b", bufs=4)
                        nc.sync.dma_start(w3b[:], w3_r[:, kc, ft * 512:(ft + 1) * 512])
                        for qt in range(QTILES):
                            nc.tensor.matmul(pss[qt][:],
                                             g[:, kc, qt * P:(qt + 1) * P],
                                             w3b[:],
                                             start=(kc == 0), stop=(kc == KC - 1))
                    for qt in range(QTILES):
                        ot = pJ.tile([P, 512], F32, tag="out_t", bufs=3)
                        nc.vector.tensor_add(ot[:], pss[qt][:],
                                             x1[:, qt, ft * 512:(ft + 1) * 512])
                        nc.sync.dma_start(out_r[:, qt, ft * 512:(ft + 1) * 512], ot[:])

    nc.compile()
    return nc


# --------------------------------------------------------------------------
# host-side prep + launch
# --------------------------------------------------------------------------

def _prep_in_maps(inputs):
    x = np.asarray(inputs["x"], np.float32)
    cos = np.asarray(inputs["cos"], np.float32)
    sin = np.asarray(inputs["sin"], np.float32)
    ln1_w = np.asarray(inputs["ln1_w"], np.float32)
    ln1_b = np.asarray(inputs["ln1_b"], np.float32)
    ln2_w = np.asarray(inputs["ln2_w"], np.float32)
    ln2_b = np.asarray(inputs["ln2_b"], np.float32)
    W_qkv = np.asarray(inputs["W_qkv"], np.float32)
    W_o = np.asarray(inputs["W_o"], np.float32)
    W1 = np.asarray(inputs["W1"], np.float32)
    W2 = np.asarray(inputs["W2"], np.float32)
    W3 = np.asarray(inputs["W3"], np.float32)

    if np.any(ln1_b):
        raise NotImplementedError("nonzero ln1_b not supported")

    Wq = (ln1_w[:, None] * W_qkv[:, :C]) / np.sqrt(np.float32(DH))
    Wk = ln1_w[:, None] * W_qkv[:, C:2 * C]
    Wv = ln1_w[:, None] * W_qkv[:, 2 * C:]
    Wkv = np.concatenate([Wk, Wv], axis=1)
    W1f = ln2_w[:, None] * W1
    W2f = ln2_w[:, None] * W2
    b1 = (ln2_b @ W1).astype(np.float32).reshape(INNER // P, P).T.copy()
    b2 = (ln2_b @ W2).astype(np.float32).reshape(INNER // P, P).T.copy()

    common = {
        "wq": Wq.astype(nbf), "wkv": Wkv.astype(nbf), "wo": W_o.astype(nbf),
        "w1": W1f.astype(nbf), "w2": W2f.astype(nbf), "w3": W3.astype(nbf),
        "cos_all": cos, "sin_all": sin, "b1v": b1, "b2v": b2,
    }
    in_maps = []
    for c in range(NCORES):
        b, i = c // 4, c % 4
        sl = slice(i * CHUNK, (i + 1) * CHUNK)
        kk = np.arange(T)[:, None]
        qq = i * CHUNK + np.arange(CHUNK)[None, :]
        m = dict(common)
        m["xb"] = np.ascontiguousarray(x[b])
        m["xown"] = np.ascontiguousarray(x[b, sl])
        m["cos_own"] = np.ascontiguousarray(cos[sl])
        m["sin_own"] = np.ascontiguousarray(sin[sl])
        m["maskT"] = (kk <= qq).astype(nbf)
        in_maps.append(m)
    return in_maps


def kernel(**inputs):
    global LAST_RESULTS
    if "nc" not in _CACHE:
        t0 = time.time()
        _CACHE["nc"] = _build_program()
        print(f"[kernel] build+compile: {time.time() - t0:.1f}s", file=sys.stderr)
    nc = _CACHE["nc"]
    in_maps = _prep_in_maps(inputs)
    trace = os.environ.get("KTRACE", "0") == "1"
    t0 = time.time()
    res = run_bass_kernel_spmd(nc, in_maps, core_ids=list(range(NCORES)),
                               trace=trace)
    print(f"[kernel] run: {time.time() - t0:.1f}s", file=sys.stderr)
    LAST_RESULTS = res
    out = np.empty((B, T, C), np.float32)
    for c in range(NCORES):
        b, i = c // 4, c % 4
        out[b, i * CHUNK:(i + 1) * CHUNK] = res.results[c]["out"]
    return out
